# revision 8
# baseline (speedup 1.0000x reference)
"""Multi-head attention (B=2, N=2048, C=1024, H=16, D=64) on 8 Trainium2
NeuronCores.

Sharding: tensor-parallel over heads x data-parallel over batch.
Core (b, g) with b in {0,1}, g in {0..3} handles batch b and heads
[4g, 4g+4). Each core computes qkv for its heads, attention, and a partial
output projection (row-parallel); the host sums the 4 partials per batch and
adds the bias.

Per-core kernel (all matmuls fp32r — fp32 storage, reduced-precision multiply,
1 PE cycle/row at >=256 moving free):
  qT/kT [d, n] via lhsT=w^T, rhs=x^T          (d on partitions, pair-packed)
  scoresT[j, i] = kT.T @ qT                   (two K=64 row-tiled matmuls)
  attnT = exp(scale * scoresT)                (ACT, PSUM->SBUF, no max pass)
  aoT[d, i] += [v | 1]^T @ attnT              (row 64 = softmax denominators)
  aoT *= 1/sums (broadcast), then out = aoT.T @ wpT partial projection.

The attention inner loop is SOFTWARE-PIPELINED one iteration deep: the PE
program order is [scores_{t+1} | av_t | filler_t] so the ACT exp latency
(~1.1us per [128,1024] tile) of iteration t is hidden under the scores and
filler matmuls of iteration t+1 — the av matmuls never wait on the exp
semaphore. The pipeline register (`pending`) is carried ACROSS chunk and
pair boundaries; the per-chunk normalize is emitted right after that
chunk's last av flush. All non-attention PE work (qkv for pair 1, the
output projection) is interleaved as lower-priority filler inside the
attention iterations, as before:
  [dma x,w | warmup] k_p0,q_p0,v -> attn p0 | qkv p1 -> attn p1 | proj(chunk)
"""
import numpy as np
import os
import sys

sys.path.insert(0, "/opt/trn_rl_repo")

B = 2
N = 2048
C = 1024
H = 16
D = 64
SCALE = D ** -0.5

HEADS_PER_CORE = 4  # 2 pairs
N_CORES = 8

_cache = {}


def _patch_ldw_opt():
    """Enable walrus's LDWEIGHTS split/prefetch pass (off in this repo's
    compile wrapper). Measured: no effect on fp32r self-loading matmuls
    (trace-identical modulo chip clock state); keep the stock pipeline."""
    if os.environ.get("K_LDWOPT") != "1" or _cache.get("ldw_patched"):
        return
    import concourse.bass_utils as bu

    orig = bu.run_command

    def patched(cmd, **kw):
        cmd = [
            c.replace("--enable-ldw-opt=false", "--enable-ldw-opt=true")
            if isinstance(c, str) else c
            for c in cmd
        ]
        return orig(cmd, **kw)

    bu.run_command = patched
    _cache["ldw_patched"] = True


def _build():
    import concourse.bass as bass
    import concourse.tile as tile
    from concourse import bacc, mybir

    _patch_ldw_opt()

    F32 = mybir.dt.float32
    F32R = mybir.dt.float32r
    BF16 = mybir.dt.bfloat16
    P = 128
    NC4 = N // 512   # 4 i-chunks of 512
    NB = N // P      # 16 n/j blocks of 128
    CO = C // P      # 8 contraction subtiles

    n_warm = int(os.environ.get("K_WARMUP", "20"))

    nc = bacc.Bacc("TRN2", target_bir_lowering=False, debug=False)
    xT = nc.dram_tensor("xT", (C, N), BF16, kind="ExternalInput")
    wqkT = nc.dram_tensor("wqkT", (C, 512), BF16, kind="ExternalInput")
    wvT = nc.dram_tensor("wvT", (C, 256), BF16, kind="ExternalInput")
    wpT = nc.dram_tensor("wpT", (256, C), BF16, kind="ExternalInput")
    out = nc.dram_tensor("out", (N, C), BF16, kind="ExternalOutput")

    with tile.TileContext(nc) as tc:
        with (
            tc.tile_pool(name="big", bufs=1) as big,
            tc.tile_pool(name="attn", bufs=3) as attn_pool,
            tc.tile_pool(name="norm", bufs=2) as norm_pool,
            tc.tile_pool(name="outp", bufs=3) as out_pool,
            tc.tile_pool(name="ps_mm", bufs=2, space="PSUM") as ps_mm,
            tc.tile_pool(name="ps_sc", bufs=2, space="PSUM") as ps_sc,
            tc.tile_pool(name="ps_av", bufs=1, space="PSUM") as ps_av,
        ):
            # ---- input DMAs: per-co (xT cols 0:512 + wqk) pairs first so
            # the first two qk chains can contract co-serially at DMA pace;
            # the rest follows consumption order.
            xT_sb = big.tile([P, CO, N], BF16)
            wqk_sb = big.tile([P, CO, 512], BF16)
            for co in range(CO):
                nc.sync.dma_start(
                    xT_sb[:, co, 0:512],
                    xT.ap()[co * P:(co + 1) * P, 0:512],
                )
                nc.sync.dma_start(
                    wqk_sb[:, co, :],
                    wqkT.ap()[co * P:(co + 1) * P, :],
                )
            wv_sb = big.tile([P, CO, 256], BF16)
            for co in range(CO):
                nc.sync.dma_start(
                    wv_sb[:, co, :],
                    wvT.ap()[co * P:(co + 1) * P, :],
                )
            for co in range(CO):
                nc.sync.dma_start(
                    xT_sb[:, co, 512:1024],
                    xT.ap()[co * P:(co + 1) * P, 512:1024],
                )
            for co in range(CO):
                nc.sync.dma_start(
                    xT_sb[:, co, 1024:2048],
                    xT.ap()[co * P:(co + 1) * P, 1024:2048],
                )
            wp_sb = big.tile([P, 2, C], BF16)
            for cs in range(2):
                nc.sync.dma_start(
                    wp_sb[:, cs, :],
                    wpT.ap()[cs * P:(cs + 1) * P, :],
                )
            ones_c = big.tile([P, 1], F32)
            nc.vector.memset(ones_c[:], 1.0)
            # Preload the exp ACT table (~2.7us) during the DMA lead-in.
            exp_warm = big.tile([P, 1], F32)
            nc.scalar.activation(
                out=exp_warm[:], in_=ones_c[:],
                func=mybir.ActivationFunctionType.Exp,
            )

            warm = big.tile([P, 512], F32R)
            nc.vector.memset(warm[:].bitcast(F32), 0.0)
            wsink = big.tile([P, 8], F32)
            for wu in range(n_warm):
                pw = ps_mm.tile([P, 512], F32, name="pwarm", tag="pm")
                nc.tensor.matmul(
                    pw[:], warm[:, 0:128], warm[:], start=True, stop=True
                )
                if wu == n_warm - 1:
                    nc.vector.tensor_copy(wsink[:], pw[:, 0:8])

            # q/k in bf16: the qk-chain PSUM->SBUF copies cast for free,
            # and the K=64 score matmuls drop from ~233ns to ~169ns avg
            # (2-byte LDWEIGHTS + bf16 stream). q,k quantization adds
            # ~4e-3 max rel err vs the 2e-2 gate.
            qk_sb = [big.tile([P, N], BF16, name=f"qk_sb{i}") for i in range(4)]
            # per-j-block v tiles (fine-grained deps so attention j=0 does not
            # wait for the whole v phase)
            v_ones = [
                big.tile([P, HEADS_PER_CORE, 65], BF16, name=f"vo{nb}")
                for nb in range(NB)
            ]
            aoT_sb = [big.tile([P, N], BF16, name=f"aoT_sb{i}") for i in range(2)]

            _qk_pending = {}

            def qk_chain_half(fc, ick, half):
                """Half of a qT/kT chain (co 0-3 or 4-7+copy)."""
                if half == 0:
                    pm = ps_mm.tile([P, 512], F32, name="pm", tag="pm")
                    _qk_pending[(fc, ick)] = pm
                else:
                    pm = _qk_pending.pop((fc, ick))
                for co in range(4 * half, 4 * half + 4):
                    nc.tensor.matmul(
                        pm[:],
                        wqk_sb[:, co, fc * P:(fc + 1) * P],
                        xT_sb[:, co, ick * 512:(ick + 1) * 512],
                        start=(co == 0),
                        stop=(co == CO - 1),
                    )
                if half == 1:
                    nc.vector.tensor_copy(
                        qk_sb[fc][:, ick * 512:(ick + 1) * 512], pm[:]
                    )

            def qk_chain(fc, ick):
                qk_chain_half(fc, ick, 0)
                qk_chain_half(fc, ick, 1)

            def v_chain(nb):
                pm = ps_mm.tile([P, 512], F32, name="pm", tag="pm")
                for co in range(CO):
                    nc.tensor.matmul(
                        pm[:, 0:256],
                        xT_sb[:, co, nb * P:(nb + 1) * P],
                        wv_sb[:, co, :],
                        start=(co == 0),
                        stop=(co == CO - 1),
                    )
                nc.vector.tensor_copy(
                    v_ones[nb][:, :, 0:64],
                    pm[:, 0:256].rearrange("p (h d) -> p h d", h=HEADS_PER_CORE),
                )
                nc.vector.tensor_copy(
                    v_ones[nb][:, :, 64:65],
                    ones_c.unsqueeze(1).to_broadcast((P, HEADS_PER_CORE, 1)),
                )

            # ---- software-pipelined attention ----
            # pending = (at, pair, ick, jb, av_A, av_B) not yet multiplied
            # into the av accumulators.
            state = {"pending": None, "av": None}

            def av_flush():
                p = state["pending"]
                if p is None:
                    return
                at, pair, ick, jb, av_A, av_B = p
                state["pending"] = None
                hA, hB = 2 * pair, 2 * pair + 1
                nc.tensor.matmul(
                    av_A[:], v_ones[jb][:, hA, :], at[:, 0, :],
                    start=(jb == 0), stop=(jb == NB - 1),
                )
                nc.tensor.matmul(
                    av_B[:], v_ones[jb][:, hB, :], at[:, 1, :],
                    start=(jb == 0), stop=(jb == NB - 1),
                )
                if jb == NB - 1:
                    normalize(pair, ick, av_A, av_B)
                    state["av"] = None

            def attn_iter(pair, ick, jb):
                """Emit scores+exp for (pair, ick, jb); returns nothing.
                The av matmuls for the PREVIOUS iteration are emitted by the
                caller via av_flush() AFTER this (pipeline order)."""
                q_t = qk_sb[2 * pair]
                k_t = qk_sb[2 * pair + 1]
                isl = slice(ick * 512, (ick + 1) * 512)
                jsl = slice(jb * P, (jb + 1) * P)
                sc = ps_sc.tile([P, 2, 512], F32, name="sc")
                nc.tensor.matmul(
                    sc[:, 0, :], k_t[0:64, jsl], q_t[0:64, isl],
                    start=True, stop=True,
                )
                nc.tensor.matmul(
                    sc[:, 1, :], k_t[64:128, jsl], q_t[64:128, isl],
                    start=True, stop=True,
                )
                at = attn_pool.tile([P, 2, 512], BF16, name="at")
                nc.scalar.activation(
                    out=at[:], in_=sc[:],
                    func=mybir.ActivationFunctionType.Exp,
                    scale=float(SCALE),
                )
                if jb == 0:
                    av_A = ps_av.tile([65, 512], F32, name="av_A")
                    av_B = ps_av.tile([65, 512], F32, name="av_B")
                    state["av"] = (av_A, av_B)
                else:
                    av_A, av_B = state["av"]
                state["pending"] = (at, pair, ick, jb, av_A, av_B)

            def normalize(pair, ick, av_A, av_B):
                """Copy unnormalized aoT + sums to SBUF (frees the av banks),
                then normalize aoT in place."""
                isl = slice(ick * 512, (ick + 1) * 512)
                sumsA = norm_pool.tile([1, 512], F32, name="sumsA")
                sumsB = norm_pool.tile([1, 512], F32, name="sumsB")
                nc.vector.tensor_copy(aoT_sb[pair][0:64, isl], av_A[0:64, :])
                nc.vector.tensor_copy(aoT_sb[pair][64:128, isl], av_B[0:64, :])
                nc.vector.tensor_copy(sumsA[:], av_A[64:65, :])
                nc.vector.tensor_copy(sumsB[:], av_B[64:65, :])
                recA = norm_pool.tile([1, 512], F32, name="recA")
                recB = norm_pool.tile([1, 512], F32, name="recB")
                nc.vector.reciprocal_approx_fast(out=recA[:], in_=sumsA[:])
                nc.vector.reciprocal_approx_fast(out=recB[:], in_=sumsB[:])
                rbcA = norm_pool.tile([64, 512], F32, name="rbcA")
                rbcBhi = norm_pool.tile([P, 512], F32, name="rbcBhi")
                nc.gpsimd.partition_broadcast(rbcA[:], recA[:])
                nc.gpsimd.partition_broadcast(rbcBhi[0:64, :], recB[:])
                # DVE SBUF+SBUF inputs must share base partition; shift head
                # B's recip rows up to partitions 64-127 first.
                nc.vector.tensor_copy(rbcBhi[64:128, :], rbcBhi[0:64, :])
                nc.vector.tensor_mul(
                    aoT_sb[pair][0:64, isl], aoT_sb[pair][0:64, isl], rbcA[:]
                )
                nc.vector.tensor_mul(
                    aoT_sb[pair][64:128, isl],
                    aoT_sb[pair][64:128, isl],
                    rbcBhi[64:128, :],
                )

            def proj_half(nb, fck, copy_eng="v"):
                nsl = slice(nb * P, (nb + 1) * P)
                fsl = slice(fck * 512, (fck + 1) * 512)
                pj = ps_mm.tile([P, 512], F32, name="pj", tag="pm")
                nc.tensor.matmul(
                    pj[:], aoT_sb[0][:, nsl], wp_sb[:, 0, fsl],
                    start=True, stop=False,
                )
                nc.tensor.matmul(
                    pj[:], aoT_sb[1][:, nsl], wp_sb[:, 1, fsl],
                    start=False, stop=True,
                )
                ot = out_pool.tile([P, 512], BF16, name="ot")
                if copy_eng == "s":
                    nc.scalar.copy(ot[:], pj[:])
                else:
                    nc.vector.tensor_copy(ot[:], pj[:])
                nc.sync.dma_start(out.ap()[nsl, fsl], ot[:])

            def proj_block(nb, split=False):
                proj_half(nb, 0, copy_eng="s" if split else "v")
                proj_half(nb, 1)

            # ---- emission: attention p0 starts after the minimal deps
            # (k chunk 0, q chunk 0, v blocks 0-1); everything else — rest of
            # k_p0/q_p0, v chains, pair-1 qkv, the projection — interleaves
            # into the attention iterations as lower-priority PE filler,
            # ordered to match the DMA arrival of the xT column ranges each
            # piece contracts (v_n is only needed by the av flush at
            # iteration n+1; k_p0 chunk c by the scores at jb=4c).
            qk_chain(1, 0)        # k_p0 cols 0:512  (j-blocks 0-3)
            qk_chain(0, 0)        # q_p0 cols 0:512  (i-chunk 0)
            # v blocks 0-7 depend only on xT cols 0:1024: run them in the
            # DMA lead window alongside the first scores/exps.
            for nb in range(8):
                v_chain(nb)

            def multi(*fns):
                def run():
                    for f in fns:
                        f()
                return run

            f00 = {
                0: [lambda: v_chain(8)],
                1: [lambda: v_chain(9)],
                2: [lambda: v_chain(10)],
                3: [lambda: qk_chain(1, 1)],
                4: [lambda: v_chain(11), lambda: v_chain(12)],
                5: [lambda: qk_chain_half(1, 2, 0)],
                6: [lambda: qk_chain_half(1, 2, 1)],
                7: [lambda: v_chain(13), lambda: v_chain(14)],
                8: [lambda: v_chain(15), lambda: qk_chain_half(0, 1, 0)],
                9: [lambda: qk_chain_half(0, 1, 1)],
                10: [lambda: qk_chain_half(1, 3, 0)],
                11: [lambda: qk_chain_half(1, 3, 1)],
                14: [lambda: qk_chain_half(0, 2, 0)],
                15: [lambda: qk_chain_half(0, 2, 1)],
            }

            fillers = {}
            fillers[(0, 0)] = {jb: multi(*fns) for jb, fns in f00.items() if fns}
            fillers[(0, 1)] = {2: lambda: qk_chain_half(0, 3, 0),
                              4: lambda: qk_chain_half(0, 3, 1),
                              7: lambda: qk_chain_half(3, 0, 0),
                              9: lambda: qk_chain_half(3, 0, 1),
                              11: lambda: qk_chain_half(3, 1, 0),
                              13: lambda: qk_chain_half(3, 1, 1)}
            fillers[(0, 2)] = {q + 2: (lambda q=q: qk_chain_half(3, 2 + q // 2, q % 2))
                              for q in range(4)}
            fillers[(0, 2)].update(
                {q + 8: (lambda q=q: qk_chain_half(2, q // 2, q % 2))
                 for q in range(4)})
            fillers[(0, 3)] = {q + 2: (lambda q=q: qk_chain_half(2, 2 + q // 2, q % 2))
                              for q in range(4)}
            # pair-1 chunk c carries proj of chunk c-1 (jb 2..9)
            for ick in range(NC4):
                fill = {}
                if ick > 0:
                    base = 4 * (ick - 1)
                    fill = {q + 2: (lambda nb=base + q // 2, f=q % 2:
                                    proj_half(nb, f))
                            for q in range(8)}
                fillers[(1, ick)] = fill

            chunk_order = [(0, 0), (0, 1), (0, 2), (0, 3),
                           (1, 0), (1, 1), (1, 2), (1, 3)]
            for (pair, ick) in chunk_order:
                fill = fillers.get((pair, ick), {})
                for jb in range(NB):
                    attn_iter(pair, ick, jb)
                    av_flush()
                    if jb in fill:
                        fill[jb]()
            av_flush()  # last chunk's jb=15 (triggers its normalize)

            # tail: proj for chunk 3 (blocks 12-15); alternate the PSUM->SBUF
            # copies between ACT and DVE so the drain parallelizes.
            for nb in range(12, 16):
                proj_block(nb, split=True)

    nc.compile()
    return nc


def _get_nc():
    if "nc" not in _cache:
        _cache["nc"] = _build()
    return _cache["nc"]


def _shard_inputs(x, w_qkv, w_proj):
    """Build per-core input dicts (bf16). Core index = b * 4 + g."""
    import ml_dtypes

    BF = ml_dtypes.bfloat16
    in_maps = []
    for b in range(B):
        xTb = np.ascontiguousarray(x[b].T.astype(BF))  # [C, N]
        for g in range(4):
            r = g * 256  # head-group row offset within each of q/k/v sections
            wqkT = np.empty((C, 512), BF)
            wqkT[:, 0:128] = w_qkv[r:r + 128].T                  # q pair 0
            wqkT[:, 128:256] = w_qkv[C + r:C + r + 128].T        # k pair 0
            wqkT[:, 256:384] = w_qkv[r + 128:r + 256].T          # q pair 1
            wqkT[:, 384:512] = w_qkv[C + r + 128:C + r + 256].T  # k pair 1
            wvT = np.ascontiguousarray(
                w_qkv[2 * C + r:2 * C + r + 256].T.astype(BF))
            wpT = np.ascontiguousarray(w_proj[:, r:r + 256].T.astype(BF))
            in_maps.append({
                "xT": xTb,
                "wqkT": wqkT,
                "wvT": wvT,
                "wpT": wpT,
            })
    return in_maps


def kernel(x, w_qkv, w_proj, b_proj, _trace=False):
    from concourse.bass_utils import run_bass_kernel_spmd

    x = np.asarray(x, dtype=np.float32)
    w_qkv = np.asarray(w_qkv, dtype=np.float32)
    w_proj = np.asarray(w_proj, dtype=np.float32)
    b_proj = np.asarray(b_proj, dtype=np.float32)

    nc = _get_nc()
    in_maps = _shard_inputs(x, w_qkv, w_proj)
    res = run_bass_kernel_spmd(
        nc, in_maps, core_ids=list(range(N_CORES)), trace=_trace
    )
    out = np.zeros((B, N, C), np.float32)
    for b in range(B):
        for g in range(4):
            out[b] += res.results[b * 4 + g]["out"].astype(np.float32)
    out += b_proj
    if _trace:
        _cache["last_exec_time_ns"] = res.exec_time_ns
        _cache["last_results"] = res
    return out

